# revision 6
# baseline (speedup 1.0000x reference)
"""CentroidInstanceLoss on 8 Trainium2 NeuronCores (Bass/Tile).

Data-parallel over points: each of the 8 cores processes N/8 = 32768 points.
Per-core segment sums (via one-hot matmuls) are combined with a
ReduceScatter; the [512, 257] centroid(+pull-weight) table is AllGathered
back; a second pass over the points computes the pull term; the push term
uses partition-rotated centroid diffs on the core owning each subbatch.
Host does only O(S*L) label bookkeeping and the final ~70-float combine.
"""

import numpy as np

import concourse.bass as bass
import concourse.bacc as bacc
import concourse.mybir as mybir
import concourse.tile as tile

f32 = mybir.dt.float32

# Problem shape (hardcoded per contract).
N_TOTAL = 262144
D = 256
S = 8
L = 64
NSEG = S * L  # 512
NCORES = 8
DELTA_V = 0.5
DELTA_D = 1.5

AluOp = mybir.AluOpType
ActFn = mybir.ActivationFunctionType


def build_nc(n_core: int):
    """Build the SPMD Bass program for one core holding n_core points."""
    assert n_core % 128 == 0
    T = n_core // 128  # point tiles per core
    G = min(8, T)      # norm-batch group size
    assert T % G == 0

    nc = bacc.Bacc(
        "TRN2", target_bir_lowering=False, debug=False, num_devices=NCORES
    )

    x_in = nc.dram_tensor("x", [n_core, D], f32, kind="ExternalInput")
    segrow_in = nc.dram_tensor("segrow", [n_core], f32, kind="ExternalInput")
    segcol_in = nc.dram_tensor("segcol", [128, T], f32, kind="ExternalInput")
    sbcol_in = nc.dram_tensor("sbcol", [128, T], f32, kind="ExternalInput")
    iota512_in = nc.dram_tensor("iota512", [128, NSEG], f32, kind="ExternalInput")
    iotapc_in = nc.dram_tensor("iotapc", [128, 4], f32, kind="ExternalInput")
    iota8_in = nc.dram_tensor("iota8", [128, S], f32, kind="ExternalInput")
    ones_in = nc.dram_tensor("ones1", [1, 128], f32, kind="ExternalInput")
    wblk_in = nc.dram_tensor("wblk", [L, 1], f32, kind="ExternalInput")
    crecip_in = nc.dram_tensor("crecip", [L, 1], f32, kind="ExternalInput")

    lpull_out = nc.dram_tensor("lpull", [S, 1], f32, kind="ExternalOutput")
    qrot_out = nc.dram_tensor("qrot", [L, L], f32, kind="ExternalOutput")

    segrow_v = segrow_in.ap().rearrange("(t i) -> t i", i=128)  # [T, 128]

    with tile.TileContext(nc) as tc:
        with (
            tc.tile_pool(name="const", bufs=1) as constp,
            tc.tile_pool(name="norm", bufs=1) as normp,
            tc.tile_pool(name="mu", bufs=1) as mup,
            tc.tile_pool(name="dram", bufs=1, space="DRAM") as dram,
            tc.tile_pool(name="x1", bufs=G + 4) as xp1,
            tc.tile_pool(name="oh", bufs=3) as ohp,
            tc.tile_pool(name="sqc", bufs=2) as sqcp,
        ):
            # ---- constants ----
            iota512_sb = constp.tile([128, NSEG], f32)
            nc.sync.dma_start(iota512_sb[:], iota512_in[:])
            iotapc_sb = constp.tile([128, 4], f32)
            nc.sync.dma_start(iotapc_sb[:], iotapc_in[:])
            iota8_sb = constp.tile([128, S], f32)
            nc.sync.dma_start(iota8_sb[:], iota8_in[:])
            ones_sb = constp.tile([1, 128], f32)
            nc.sync.dma_start(ones_sb[:], ones_in[:])
            segcol_sb = constp.tile([128, T], f32)
            nc.sync.dma_start(segcol_sb[:], segcol_in[:])
            sbcol_sb = constp.tile([128, T], f32)
            nc.sync.dma_start(sbcol_sb[:], sbcol_in[:])
            wblk_sb = constp.tile([L, 1], f32)
            nc.sync.dma_start(wblk_sb[:], wblk_in[:])
            crecip_sb = constp.tile([L, 1], f32)
            nc.sync.dma_start(crecip_sb[:], crecip_in[:])
            negdv_sb = constp.tile([128, 1], f32)
            nc.vector.memset(negdv_sb[:], -DELTA_V)

            ss_all = normp.tile([128, T], f32)  # sum of squares per point
            rr_all = normp.tile([128, T], f32)  # 1/(norm+eps) per point

            # ---- pass 1: per-core segment sums of normalized points ----
            with tc.tile_pool(name="psum1", bufs=1, space="PSUM") as psum1:
                ps_sums = [
                    psum1.tile([128, D], f32, tag=f"sums{c}", name=f"ps_sums{c}")
                    for c in range(4)
                ]
                for g in range(T // G):
                    xts = []
                    for j in range(G):
                        t = g * G + j
                        xt = xp1.tile([128, D], f32, tag="x1t")
                        nc.sync.dma_start(xt[:], x_in[t * 128:(t + 1) * 128, :])
                        xts.append(xt)
                        sink = sqcp.tile([128, D], f32, tag="sq_sink")
                        nc.scalar.activation(
                            sink[:], xt[:], ActFn.Square,
                            accum_out=ss_all[:, t:t + 1],
                        )
                    sqc = sqcp.tile([128, G], f32, tag="sqc")
                    nc.scalar.activation(
                        sqc[:], ss_all[:, g * G:(g + 1) * G], ActFn.Sqrt
                    )
                    nc.vector.tensor_scalar_add(sqc[:], sqc[:], 1e-8)
                    nc.vector.reciprocal(rr_all[:, g * G:(g + 1) * G], sqc[:])
                    for j in range(G):
                        t = g * G + j
                        oh = ohp.tile([128, NSEG], f32, tag="oh")
                        nc.gpsimd.tensor_scalar(
                            oh[:], iota512_sb[:],
                            segcol_sb[:, t:t + 1], rr_all[:, t:t + 1],
                            op0=AluOp.is_equal, op1=AluOp.mult,
                        )
                        for c in range(4):
                            nc.tensor.matmul(
                                ps_sums[c][:],
                                oh[:, c * 128:(c + 1) * 128],
                                xts[j][:],
                                start=(t == 0), stop=(t == T - 1),
                            )

                rs_in = dram.tile([NSEG, D], f32)
                for c in range(4):
                    sums_sb = sqcp.tile([128, D], f32, tag="sums_sb", name="sums_sb")
                    nc.vector.tensor_copy(sums_sb[:], ps_sums[c][:])
                    nc.sync.dma_start(
                        rs_in[c * 128:(c + 1) * 128, :], sums_sb[:]
                    )

            # ---- combine centroid table across cores ----
            rs_out = dram.tile([L, D], f32)
            nc.gpsimd.collective_compute(
                "ReduceScatter", AluOp.add,
                replica_groups=[list(range(NCORES))],
                ins=[rs_in.opt()], outs=[rs_out.opt()],
            )
            musb_raw = mup.tile([L, D], f32)
            nc.sync.dma_start(musb_raw[:], rs_out[:])
            muaug = mup.tile([L, D + 1], f32)
            nc.vector.tensor_scalar(
                muaug[:, 0:D], musb_raw[:], crecip_sb[:, 0:1], None,
                op0=AluOp.mult,
            )
            nc.vector.tensor_copy(muaug[:, D:D + 1], wblk_sb[:])
            ag_in = dram.tile([L, D + 1], f32)
            nc.sync.dma_start(ag_in[:], muaug[:])
            ag_out = dram.tile([NSEG, D + 1], f32, addr_space="Shared")
            nc.gpsimd.collective_compute(
                "AllGather", AluOp.bypass,
                replica_groups=[list(range(NCORES))],
                ins=[ag_in.opt()], outs=[ag_out.opt()],
            )
            mut_sb = mup.tile([128, 4, D + 1], f32)
            nc.sync.dma_start(
                mut_sb[:], ag_out.rearrange("(c p) d -> p c d", p=128)
            )

            # ---- push: pairwise centroid L1 distances (own subbatch) ----
            q_sb = mup.tile([L, L], f32)
            nc.vector.memset(q_sb[:, 0:1], 0.0)
            with (
                tc.tile_pool(name="rot", bufs=3) as rotp,
                tc.tile_pool(name="pdiff", bufs=3) as pdp,
            ):
                for k in range(1, L):
                    rot = rotp.tile([L, D], f32, tag="rot")
                    nc.sync.dma_start(rot[0:L - k, :], muaug[k:L, 0:D])
                    nc.sync.dma_start(rot[L - k:L, :], muaug[0:k, 0:D])
                    pdiff = pdp.tile([L, D], f32, tag="pdiff")
                    nc.vector.tensor_sub(pdiff[:], muaug[:, 0:D], rot[:])
                    psink = pdp.tile([L, D], f32, tag="psink")
                    nc.scalar.activation(
                        psink[:], pdiff[:], ActFn.Abs,
                        accum_out=q_sb[:, k:k + 1],
                    )
            nc.sync.dma_start(qrot_out[:], q_sb[:])

            # ---- pass 2: pull term ----
            with (
                tc.tile_pool(name="x2", bufs=4) as xp2,
                tc.tile_pool(name="srow", bufs=4) as srowp,
                tc.tile_pool(name="bcps", bufs=2, space="PSUM") as bcpsp,
                tc.tile_pool(name="bcsb", bufs=3) as bcsbp,
                tc.tile_pool(name="oht", bufs=3) as ohtp,
                tc.tile_pool(name="mups", bufs=2, space="PSUM") as mupsp,
                tc.tile_pool(name="pullps", bufs=1, space="PSUM") as pullpsp,
                tc.tile_pool(name="diff", bufs=3) as diffp,
                tc.tile_pool(name="sink2", bufs=2) as sink2p,
                tc.tile_pool(name="small", bufs=4) as smallp,
            ):
                ps_pull = pullpsp.tile([S, 1], f32)
                for t in range(T):
                    xt = xp2.tile([128, D], f32, tag="x2t")
                    nc.sync.dma_start(xt[:], x_in[t * 128:(t + 1) * 128, :])
                    srow = srowp.tile([1, 128], f32, tag="srow")
                    nc.sync.dma_start(srow[:], segrow_v[t:t + 1, :])
                    ps_bc = bcpsp.tile([128, 128], f32, tag="bc")
                    nc.tensor.matmul(
                        ps_bc[:], ones_sb[:], srow[:], start=True, stop=True
                    )
                    bc_sb = bcsbp.tile([128, 128], f32, tag="bcsb")
                    nc.vector.tensor_copy(bc_sb[:], ps_bc[:])
                    oht = ohtp.tile([128, NSEG], f32, tag="oht")
                    for c in range(4):
                        nc.gpsimd.tensor_scalar(
                            oht[:, c * 128:(c + 1) * 128], bc_sb[:],
                            iotapc_sb[:, c:c + 1], None,
                            op0=AluOp.is_equal,
                        )
                    ps_mu = mupsp.tile([128, D + 1], f32, tag="mu")
                    for c in range(4):
                        nc.tensor.matmul(
                            ps_mu[:],
                            oht[:, c * 128:(c + 1) * 128],
                            mut_sb[:, c, :],
                            start=(c == 0), stop=(c == 3),
                        )
                    diff = diffp.tile([128, D], f32, tag="diff")
                    nc.vector.scalar_tensor_tensor(
                        diff[:], xt[:], rr_all[:, t:t + 1], ps_mu[:, 0:D],
                        op0=AluOp.mult, op1=AluOp.subtract,
                    )
                    sink = sink2p.tile([128, D], f32, tag="sink2")
                    d1 = smallp.tile([128, 1], f32, tag="d1")
                    nc.scalar.activation(
                        sink[:], diff[:], ActFn.Abs, accum_out=d1[:]
                    )
                    t1 = smallp.tile([128, 1], f32, tag="t1")
                    nc.scalar.activation(
                        t1[:], d1[:], ActFn.Relu, bias=negdv_sb[:]
                    )
                    t2 = smallp.tile([128, 1], f32, tag="t2")
                    nc.vector.tensor_mul(t2[:], t1[:], t1[:])
                    v = smallp.tile([128, 1], f32, tag="v")
                    nc.vector.tensor_mul(v[:], t2[:], ps_mu[:, D:D + 1])
                    ohsb = smallp.tile([128, S], f32, tag="ohsb")
                    nc.gpsimd.tensor_scalar(
                        ohsb[:], iota8_sb[:], sbcol_sb[:, t:t + 1], None,
                        op0=AluOp.is_equal,
                    )
                    nc.tensor.matmul(
                        ps_pull[:], ohsb[:], v[:],
                        start=(t == 0), stop=(t == T - 1),
                    )
                lpull_sb = smallp.tile([S, 1], f32, tag="lpull_sb")
                nc.vector.tensor_copy(lpull_sb[:], ps_pull[:])
                nc.sync.dma_start(lpull_out[:], lpull_sb[:])

    nc.compile()
    return nc


def host_tables(labels: np.ndarray, subbatch: np.ndarray):
    """Everything derivable from the integer inputs alone."""
    seg = (subbatch.astype(np.int64) * L + labels.astype(np.int64)).astype(np.int32)
    counts = np.bincount(seg, minlength=NSEG).astype(np.float64)  # [512]
    present = counts > 0
    M = present.reshape(S, L).sum(axis=1).astype(np.float64)  # [S]
    valid = M > 1.0
    # per-seg pull weight: valid(sb)/(M_sb * count_s); 0 for invalid sb
    M_per_seg = np.repeat(M, L)
    valid_per_seg = np.repeat(valid, L)
    w = np.where(
        valid_per_seg, 1.0 / (M_per_seg * np.maximum(counts, 1.0)), 0.0
    ).astype(np.float32)
    crecip = (1.0 / np.maximum(counts, 1.0)).astype(np.float32)
    return seg, counts, present, M, valid, w, crecip


def make_in_maps(outputs: np.ndarray, labels: np.ndarray, subbatch: np.ndarray):
    n = outputs.shape[0]
    n_core = n // NCORES
    T = n_core // 128
    seg, counts, present, M, valid, w, crecip = host_tables(labels, subbatch)
    segf = seg.astype(np.float32)
    sbf = subbatch.astype(np.float32)

    iota512 = np.broadcast_to(
        np.arange(NSEG, dtype=np.float32), (128, NSEG)
    ).copy()
    iotapc = (
        np.arange(4, dtype=np.float32)[None, :] * 128.0
        + np.arange(128, dtype=np.float32)[:, None]
    ).copy()  # [128, 4]
    iota8 = np.broadcast_to(np.arange(S, dtype=np.float32), (128, S)).copy()
    ones1 = np.ones((1, 128), dtype=np.float32)

    in_maps = []
    for c in range(NCORES):
        sl = slice(c * n_core, (c + 1) * n_core)
        segc = segf[sl]
        sbc = sbf[sl]
        blk = slice(c * L, (c + 1) * L)
        in_maps.append({
            "x": np.ascontiguousarray(outputs[sl]),
            "segrow": segc,
            "segcol": np.ascontiguousarray(segc.reshape(T, 128).T),
            "sbcol": np.ascontiguousarray(sbc.reshape(T, 128).T),
            "iota512": iota512,
            "iotapc": iotapc,
            "iota8": iota8,
            "ones1": ones1,
            "wblk": w[blk].reshape(L, 1),
            "crecip": crecip[blk].reshape(L, 1),
        })
    return in_maps, (seg, counts, present, M, valid, w, crecip)


def combine(results, tables, n: int):
    """Host combine of the per-core outputs into the scalar loss."""
    seg, counts, present, M, valid, w, crecip = tables
    pull_total = np.float64(0.0)
    for r in results:
        pull_total += r["lpull"].astype(np.float64).sum()

    push_total = np.float64(0.0)
    pres_sl = present.reshape(S, L)
    for sb in range(S):
        if not valid[sb]:
            continue
        q = results[sb]["qrot"].astype(np.float64)  # [64(a), 64(k)]
        a = np.arange(L)
        dist = np.zeros((L, L))
        for k in range(1, L):
            dist[a, (a + k) % L] = q[:, k]
        p = pres_sl[sb]
        mask = p[:, None] & p[None, :] & ~np.eye(L, dtype=bool)
        r = np.maximum(2.0 * DELTA_D - dist, 0.0) ** 2
        push = np.where(mask, r, 0.0).sum()
        push_total += push / max(M[sb] * (M[sb] - 1.0), 1.0)

    return np.float32((pull_total + push_total) / n)


_NC_CACHE: dict = {}


def _get_nc(n_core: int):
    if n_core not in _NC_CACHE:
        _NC_CACHE[n_core] = build_nc(n_core)
    return _NC_CACHE[n_core]


def kernel(outputs, labels, subbatch_indices):
    from concourse.bass_utils import run_bass_kernel_spmd

    outputs = np.asarray(outputs, dtype=np.float32)
    labels = np.asarray(labels, dtype=np.int32)
    subbatch_indices = np.asarray(subbatch_indices, dtype=np.int32)
    n = outputs.shape[0]
    n_core = n // NCORES

    nc = _get_nc(n_core)
    in_maps, tables = make_in_maps(outputs, labels, subbatch_indices)
    res = run_bass_kernel_spmd(nc, in_maps, list(range(NCORES)))
    return combine(res.results, tables, n)
